# revision 9
# baseline (speedup 1.0000x reference)
"""Trainium2 Bass kernel for the EnhancedMamba2Mixer problem.

Sharding: 2-way data parallel over batch x 4-way tensor parallel over heads
(16 heads / 1024 INTER channels per core).  All 8 cores run one identical
Bass program on different input slices; no collectives.  The RMSNorm rsqrt
is a per-row scalar that commutes with the linear out_proj, so each core
emits a partial out_proj product plus its local gated activations z; the
host sums partials, computes the variance from z, and applies the scale.

Device program per core (chunked SSD scan, chunk L=128):
  in_proj (gate|hs columns, bf16) -> causal depthwise conv + SiLU
  -> per chunk: G^T = B @ C^T (shared across heads since n_groups=1),
     per head: M^T = G^T * E^T (host-precomputed decay mask),
     Y^T = X~^T M + (S_prev C^T) * exp(cum), state' = dA*state + B^T X2
  -> z = y * silu(gate) -> partial out_proj.
Small exp/softplus/cumsum tensors (0.1% of FLOPs) are prepared on host in
fp64 to protect the exponential-path precision.
"""
import sys

sys.path.insert(0, "/opt/trn_rl_repo")

from contextlib import ExitStack

import ml_dtypes
import numpy as np

import concourse.bass as bass  # noqa: F401  (engine types)
import concourse.mybir as mybir
import concourse.tile as tile
from concourse import bacc
from concourse.masks import make_identity

HID = 2048
INTER = 4096
NH = 64
HD = 64
NST = 128          # ssm state size
KCV = 4            # conv kernel
EPS = 1e-5
B = 2
S = 2048
L = 128            # scan chunk length
NCHUNK = S // L    # 16
NCORES = 8
HLOC = 16          # heads per core
CLOC = 1024        # INTER channels per core
HALF = S // 2

BF16 = mybir.dt.bfloat16
F32 = mybir.dt.float32
bfnp = ml_dtypes.bfloat16
MUL = mybir.AluOpType.mult
ADD = mybir.AluOpType.add
SIGM = mybir.ActivationFunctionType.Sigmoid

_CACHE = {}


def _build_program():
    nc = bacc.Bacc("TRN2", target_bir_lowering=False, debug=False,
                   num_devices=NCORES)

    def din(name, shape, dt):
        return nc.dram_tensor(name, shape, dt, kind="ExternalInput").ap()

    XT = din("xt", [HID, S], BF16)              # x transposed [hid, seq]
    WT = din("wt", [HID, 2048], BF16)           # W^T cols: 0:1024 gate, 1024:2048 hs
    WO = din("wo", [CLOC, HID], BF16)           # (Wout*normw)^T  [j, o]
    BT = din("bt", [NST, S], BF16)              # B^T post conv+silu
    CT = din("ct", [NST, S], BF16)              # C^T post conv+silu
    BR = din("br", [S, NST], BF16)              # B rows
    ETD = din("et", [NCHUNK, HLOC, L, L], BF16)     # decay mask E^T [s,t]
    EXC = din("exc", [NCHUNK, 8, L, L], BF16)       # exp(cum_t), pair-row-replicated
    XSC = din("xsc", [NCHUNK, HLOC, L, 2], F32)     # dt, dt*exp(cumL-cum) per s
    DAC = din("dac", [128, NCHUNK * HLOC], F32)     # exp(cumL) replicated
    DCO = din("dco", [128, 8], F32)                 # D per pair, row-split
    CW = din("cw", [8, 128, KCV], F32)              # conv weights per ch tile
    CB = din("cb", [8, 128], F32)                   # conv bias
    OUTT = nc.dram_tensor("outt", [HID, S], F32, kind="ExternalOutput").ap()
    ZT = nc.dram_tensor("ztout", [CLOC, S], BF16, kind="ExternalOutput").ap()

    with tile.TileContext(nc) as tc, ExitStack() as ctx:
        P = ctx.enter_context
        const = P(tc.tile_pool(name="const", bufs=1))
        wpool = P(tc.tile_pool(name="wpool", bufs=2))
        wopool = P(tc.tile_pool(name="wopool", bufs=2))
        xpool = P(tc.tile_pool(name="xpool", bufs=1))
        hpool = P(tc.tile_pool(name="hpool", bufs=1))
        sgpool = P(tc.tile_pool(name="sgpool", bufs=2))
        ztpool = P(tc.tile_pool(name="ztpool", bufs=2))
        etpool = P(tc.tile_pool(name="etpool", bufs=2))
        brpool = P(tc.tile_pool(name="brpool", bufs=2))
        scr = P(tc.tile_pool(name="scr", bufs=3))
        cscr = P(tc.tile_pool(name="cscr", bufs=2))
        oev = P(tc.tile_pool(name="oev", bufs=3))
        mm_ps = P(tc.tile_pool(name="mm_ps", bufs=3, space="PSUM"))
        g_ps = P(tc.tile_pool(name="g_ps", bufs=1, space="PSUM"))
        xt_ps = P(tc.tile_pool(name="xt_ps", bufs=1, space="PSUM"))
        y_ps = P(tc.tile_pool(name="y_ps", bufs=1, space="PSUM"))
        z_ps = P(tc.tile_pool(name="z_ps", bufs=1, space="PSUM"))
        st_ps = P(tc.tile_pool(name="st_ps", bufs=1, space="PSUM"))

        id128 = const.tile([128, 128], BF16)
        make_identity(nc, id128[:])
        xsc_s = const.tile([128, NCHUNK, HLOC, 2], F32)
        nc.sync.dma_start(xsc_s[:], XSC.rearrange("c h s j -> s c h j"))
        dac_s = const.tile([128, NCHUNK * HLOC], F32)
        nc.sync.dma_start(dac_s[:], DAC)
        dco_s = const.tile([128, 8], F32)
        nc.sync.dma_start(dco_s[:], DCO)
        cw_s = const.tile([128, 8, KCV], F32)
        nc.sync.dma_start(cw_s[:], CW.rearrange("j p k -> p j k"))
        cb_s = const.tile([128, 8], F32)
        nc.sync.dma_start(cb_s[:], CB.rearrange("j p -> p j"))
        bt_s = const.tile([128, S], BF16)
        nc.sync.dma_start(bt_s[:], BT)
        ct_s = const.tile([128, S], BF16)
        nc.sync.dma_start(ct_s[:], CT)
        state = const.tile([128, HLOC, HD], BF16)
        nc.gpsimd.memset(state[:], 0.0)
        carry = const.tile([128, 8, 3], BF16)
        nc.gpsimd.memset(carry[:], 0.0)

        for hf in range(2):
            q0 = hf * HALF
            xh = xpool.tile([128, 16, HALF], BF16, tag="xh")
            nc.sync.dma_start(
                xh[:], XT[:, q0:q0 + HALF].rearrange("(k p) t -> p k t", p=128))
            sg = sgpool.tile([128, 8, HALF], BF16, tag="sg")
            hraw = hpool.tile([128, 8, HALF + 3], BF16, tag="hraw")
            hconv = hpool.tile([128, 8, HALF], BF16, tag="hconv")
            zts = ztpool.tile([128, 8, HALF], BF16, tag="zts")

            # ---- in_proj ----
            for j in range(16):
                wst = wpool.tile([128, 16, 128], BF16, tag="wst")
                nc.sync.dma_start(
                    wst[:],
                    WT[:, j * 128:(j + 1) * 128].rearrange("(k p) c -> p k c",
                                                           p=128))
                for s2 in range(2):
                    ps = mm_ps.tile([128, 512], F32, tag="mmps")
                    for k in range(16):
                        nc.tensor.matmul(
                            ps[:], wst[:, k, :],
                            xh[:, k, s2 * 512:(s2 + 1) * 512],
                            start=(k == 0), stop=(k == 15))
                    dst = slice(s2 * 512, (s2 + 1) * 512)
                    if j < 8:
                        sig = cscr.tile([128, 512], BF16, tag="sig")
                        nc.scalar.activation(sig[:], ps[:], SIGM)
                        nc.vector.tensor_mul(sg[:, j, dst], ps[:], sig[:])
                    else:
                        nc.scalar.copy(hraw[:, j - 8, 3 + s2 * 512:
                                            3 + (s2 + 1) * 512], ps[:])

            # ---- conv (carry in from previous half) ----
            nc.vector.tensor_copy(hraw[:, :, 0:3], carry[:])
            if hf == 0:
                nc.vector.tensor_copy(carry[:], hraw[:, :, HALF:HALF + 3])
            for hj in range(8):
                a1 = cscr.tile([128, HALF], F32, tag="cacc1")
                nc.vector.tensor_scalar(a1[:], hraw[:, hj, 0:HALF],
                                        cw_s[:, hj, 0:1], cb_s[:, hj:hj + 1],
                                        MUL, ADD)
                a2 = cscr.tile([128, HALF], F32, tag="cacc2")
                nc.vector.scalar_tensor_tensor(a2[:], hraw[:, hj, 1:HALF + 1],
                                               cw_s[:, hj, 1:2], a1[:], MUL, ADD)
                a3 = cscr.tile([128, HALF], F32, tag="cacc1")
                nc.vector.scalar_tensor_tensor(a3[:], hraw[:, hj, 2:HALF + 2],
                                               cw_s[:, hj, 2:3], a2[:], MUL, ADD)
                a4 = cscr.tile([128, HALF], F32, tag="cacc2")
                nc.vector.scalar_tensor_tensor(a4[:], hraw[:, hj, 3:HALF + 3],
                                               cw_s[:, hj, 3:4], a3[:], MUL, ADD)
                sig4 = cscr.tile([128, HALF], BF16, tag="csig")
                nc.scalar.activation(sig4[:], a4[:], SIGM)
                nc.vector.tensor_mul(hconv[:, hj, :], a4[:], sig4[:])

            # ---- scan ----
            for cl in range(8):
                cg = hf * 8 + cl
                t0 = cg * 128
                tl = slice(cl * 128, (cl + 1) * 128)
                et = etpool.tile([128, HLOC, 128], BF16, tag="et")
                nc.sync.dma_start(et[:], ETD[cg].rearrange("h s t -> s h t"))
                exc = etpool.tile([128, 8, 128], BF16, tag="exc")
                nc.sync.dma_start(exc[:], EXC[cg].rearrange("p d t -> d p t"))
                br = brpool.tile([128, 128], BF16, tag="br")
                nc.sync.dma_start(br[:], BR[t0:t0 + 128, :])
                gps = g_ps.tile([128, 128], F32, tag="gps")
                nc.tensor.matmul(gps[:], bt_s[:, t0:t0 + 128],
                                 ct_s[:, t0:t0 + 128], start=True, stop=True)
                for p in range(8):
                    heads = (2 * p, 2 * p + 1)
                    xtp = xt_ps.tile([128, 128], BF16, tag="xtps")
                    nc.tensor.transpose(xtp[:], hconv[:, p, tl], id128[:])
                    xpair = scr.tile([128, 128], BF16, tag="xpair")
                    x2pair = scr.tile([128, 128], BF16, tag="x2pair")
                    for hh, hl in enumerate(heads):
                        dsl = slice(hh * 64, (hh + 1) * 64)
                        nc.vector.tensor_scalar_mul(
                            xpair[:, dsl], xtp[:, dsl], xsc_s[:, cg, hl, 0:1])
                        nc.vector.tensor_scalar_mul(
                            x2pair[:, dsl], xtp[:, dsl], xsc_s[:, cg, hl, 1:2])
                    yps = y_ps.tile([128, 128], F32, tag="yps")
                    zps = z_ps.tile([128, 128], F32, tag="zps")
                    stp = st_ps.tile([128, 128], F32, tag="stps")
                    mts = []
                    for hh, hl in enumerate(heads):
                        mt = scr.tile([128, 128], BF16, tag=f"mt{hh}")
                        nc.vector.tensor_mul(mt[:], gps[:], et[:, hl, :])
                        mts.append(mt)
                    for hh, hl in enumerate(heads):
                        dsl = slice(hh * 64, (hh + 1) * 64)
                        nc.tensor.matmul(yps[dsl, :], xpair[:, dsl],
                                         mts[hh][:], start=True, stop=True)
                        nc.tensor.matmul(zps[dsl, :], state[:, hl, :],
                                         ct_s[:, t0:t0 + 128],
                                         start=True, stop=True)
                        nc.tensor.matmul(stp[:, dsl], br[:], x2pair[:, dsl],
                                         start=True, stop=True)
                    for hh, hl in enumerate(heads):
                        dsl = slice(hh * 64, (hh + 1) * 64)
                        idx = cg * HLOC + hl
                        nc.vector.scalar_tensor_tensor(
                            state[:, hl, :], state[:, hl, :],
                            dac_s[:, idx:idx + 1], stp[:, dsl], MUL, ADD)
                    t1 = scr.tile([128, 128], F32, tag="t1")
                    nc.vector.tensor_mul(t1[:], zps[:], exc[:, p, :])
                    t2 = scr.tile([128, 128], F32, tag="t2")
                    nc.vector.scalar_tensor_tensor(
                        t2[:], hconv[:, p, tl], dco_s[:, p:p + 1], yps[:],
                        MUL, ADD)
                    yv = scr.tile([128, 128], F32, tag="yv")
                    nc.vector.tensor_add(yv[:], t1[:], t2[:])
                    nc.vector.tensor_mul(zts[:, p, tl], yv[:], sg[:, p, tl])

            # ---- out_proj ----
            for o in range(16):
                wo = wopool.tile([128, 8, 128], BF16, tag="wo")
                nc.sync.dma_start(
                    wo[:],
                    WO[:, o * 128:(o + 1) * 128].rearrange("(kj p) c -> p kj c",
                                                           p=128))
                for s2 in range(2):
                    ps = mm_ps.tile([128, 512], F32, tag="mmps")
                    for kj in range(8):
                        nc.tensor.matmul(
                            ps[:], wo[:, kj, :],
                            zts[:, kj, s2 * 512:(s2 + 1) * 512],
                            start=(kj == 0), stop=(kj == 7))
                    ov = oev.tile([128, 512], F32, tag="oev")
                    nc.vector.tensor_copy(ov[:], ps[:])
                    nc.sync.dma_start(
                        OUTT[o * 128:(o + 1) * 128,
                             q0 + s2 * 512:q0 + (s2 + 1) * 512], ov[:])

            nc.sync.dma_start(
                ZT[:, q0:q0 + HALF].rearrange("(j p) t -> p j t", p=128),
                zts[:])

    nc.compile()
    return nc


def _softplus64(x):
    x = np.asarray(x, np.float64)
    return np.where(x > 30, x, np.log1p(np.exp(np.minimum(x, 30.0))))


def _silu(x):
    return x / (1.0 + np.exp(-x))


def _causal_conv(x, w, b):
    # x [S, C], w [C, K], b [C]
    pad = np.zeros((KCV - 1, x.shape[1]), x.dtype)
    xp = np.concatenate([pad, x], axis=0)
    out = np.zeros_like(x)
    for k in range(KCV):
        out += xp[k:k + x.shape[0]] * w[None, :, k]
    return out + b[None, :]


def _host_prep(inputs):
    hs = np.asarray(inputs["hidden_states"], np.float32)
    W = np.asarray(inputs["in_proj_w"], np.float32)
    cw = np.asarray(inputs["conv_w"], np.float32)[:, 0, :]
    cb = np.asarray(inputs["conv_b"], np.float32)
    dt_bias = np.asarray(inputs["dt_bias"], np.float64)
    A = -np.exp(np.asarray(inputs["A_log"], np.float64))
    D = np.asarray(inputs["D"], np.float32)
    nw = np.asarray(inputs["norm_weight"], np.float32)
    Wout = np.asarray(inputs["out_proj_w"], np.float32)

    Wg = W[0:INTER]
    Whs = W[INTER:2 * INTER]
    Wbc = W[2 * INTER:2 * INTER + 2 * NST]
    Wdt = W[2 * INTER + 2 * NST:]

    per_batch = []
    for b in range(B):
        x = hs[b]
        bc_raw = x @ Wbc.T
        dt_raw = x @ Wdt.T
        bc = _silu(_causal_conv(bc_raw, cw[INTER:], cb[INTER:]))
        Bm = bc[:, :NST].astype(np.float32)
        Cm = bc[:, NST:].astype(np.float32)
        dt = _softplus64(dt_raw.astype(np.float64) + dt_bias[None, :])
        cum = (dt * A[None, :]).reshape(NCHUNK, L, NH).cumsum(axis=1)  # f64
        per_batch.append((x, Bm, Cm, dt, cum))

    sidx = np.arange(L)
    causal = (sidx[None, :] >= sidx[:, None])  # [s, t] keep t >= s
    in_maps = []
    for core in range(NCORES):
        b, tp = divmod(core, 4)
        x, Bm, Cm, dt, cum = per_batch[b]
        csel = slice(tp * CLOC, (tp + 1) * CLOC)
        h0 = tp * HLOC
        cuml = cum[:, :, h0:h0 + HLOC]          # [c, i, 16] f64
        dtl = dt[:, h0:h0 + HLOC].reshape(NCHUNK, L, HLOC)

        # E^T mask [c, h, s, t]
        diff = cuml[:, None, :, :] - cuml[:, :, None, :]    # [c, s, t, h]? no:
        # cuml[c, t, h] - cuml[c, s, h] with axes [c, s, t, h]
        et = np.exp(np.where(causal[None, :, :, None], diff, -np.inf))
        et = np.transpose(et, (0, 3, 1, 2)).astype(bfnp)    # [c, h, s, t]

        expc = np.exp(cuml)                                  # [c, t, h]
        exc = np.empty((NCHUNK, 8, L, L), np.float32)
        for p in range(8):
            exc[:, p, 0:64, :] = expc[:, None, :, 2 * p]     # bcast rows
            exc[:, p, 64:128, :] = expc[:, None, :, 2 * p + 1]
        exc = exc.astype(bfnp)

        xsc = np.empty((NCHUNK, HLOC, L, 2), np.float32)
        xsc[..., 0] = np.transpose(dtl, (0, 2, 1))
        x2s = dtl * np.exp(cuml[:, -1:, :] - cuml)           # [c, i, h]
        xsc[..., 1] = np.transpose(x2s, (0, 2, 1))

        dac = np.broadcast_to(
            np.exp(cuml[:, -1, :]).reshape(1, NCHUNK * HLOC),
            (128, NCHUNK * HLOC)).astype(np.float32).copy()

        dco = np.empty((128, 8), np.float32)
        for p in range(8):
            dco[0:64, p] = D[h0 + 2 * p]
            dco[64:128, p] = D[h0 + 2 * p + 1]

        wt = np.concatenate([Wg[csel], Whs[csel]], axis=0).T.copy()  # [hid, 2048]
        wo = (Wout[:, csel] * nw[None, csel]).T.copy()               # [j, o]

        in_maps.append({
            "xt": np.ascontiguousarray(x.T).astype(bfnp),
            "wt": wt.astype(bfnp),
            "wo": wo.astype(bfnp),
            "bt": np.ascontiguousarray(Bm.T).astype(bfnp),
            "ct": np.ascontiguousarray(Cm.T).astype(bfnp),
            "br": Bm.astype(bfnp),
            "et": np.ascontiguousarray(et),
            "exc": np.ascontiguousarray(exc),
            "xsc": xsc,
            "dac": dac,
            "dco": dco,
            "cw": np.ascontiguousarray(
                cw[csel].reshape(8, 128, KCV)),
            "cb": np.ascontiguousarray(cb[csel].reshape(8, 128)),
        })
    return in_maps


def _get_runner(nc):
    """Cached jitted SPMD runner (mirrors bass2jax.run_bass_via_pjrt)."""
    if "runner" in _CACHE:
        return _CACHE["runner"]
    import jax
    from jax.sharding import Mesh, PartitionSpec
    from jax.experimental.shard_map import shard_map
    from concourse import bass2jax

    bass2jax.install_neuronx_cc_hook()
    partition_name = (nc.partition_id_tensor.name
                      if nc.partition_id_tensor else None)
    in_names, out_names, out_avals, zero_shapes = [], [], [], []
    for alloc in nc.m.functions[0].allocations:
        if not isinstance(alloc, mybir.MemoryLocationSet):
            continue
        name = alloc.memorylocations[0].name
        if alloc.kind == "ExternalInput":
            if name != partition_name:
                in_names.append(name)
        elif alloc.kind == "ExternalOutput":
            out_names.append(name)
            shape = tuple(alloc.tensor_shape)
            dtype = mybir.dt.np(alloc.dtype)
            out_avals.append(jax.core.ShapedArray(shape, dtype))
            zero_shapes.append((shape, dtype))
    n_params = len(in_names)
    all_in_names = in_names + out_names
    if partition_name is not None:
        all_in_names = all_in_names + [partition_name]
    donate = tuple(range(n_params, n_params + len(out_names)))

    def _body(*args):
        operands = list(args)
        if partition_name is not None:
            operands.append(bass2jax.partition_id_tensor())
        outs = bass2jax._bass_exec_p.bind(
            *operands,
            out_avals=tuple(out_avals),
            in_names=tuple(all_in_names),
            out_names=tuple(out_names),
            lowering_input_output_aliases=(),
            sim_require_finite=True,
            sim_require_nnan=True,
            nc=nc,
        )
        return tuple(outs)

    devices = jax.devices()[:NCORES]
    mesh = Mesh(np.asarray(devices), ("core",))
    specs = (PartitionSpec("core"),) * (n_params + len(out_names))
    sharded = jax.jit(
        shard_map(_body, mesh=mesh, in_specs=specs,
                  out_specs=(PartitionSpec("core"),) * len(out_names),
                  check_rep=False),
        donate_argnums=donate, keep_unused=True)

    def run(in_maps):
        concat_in = [
            np.concatenate([np.asarray(m[name]) for m in in_maps], axis=0)
            for name in in_names
        ]
        concat_zeros = [
            np.zeros((NCORES * sh[0],) + sh[1:], dt) for sh, dt in zero_shapes
        ]
        out_arrs = sharded(*concat_in, *concat_zeros)
        return [
            {name: np.asarray(out_arrs[i]).reshape(
                (NCORES,) + zero_shapes[i][0])[c]
             for i, name in enumerate(out_names)}
            for c in range(NCORES)
        ]

    _CACHE["runner"] = run
    return run


def kernel(**inputs):
    if "nc" not in _CACHE:
        _CACHE["nc"] = _build_program()
    nc = _CACHE["nc"]
    in_maps = _host_prep(inputs)
    results = _get_runner(nc)(in_maps)

    out = np.zeros((B, S, HID), np.float32)
    for b in range(B):
        acc = np.zeros((HID, S), np.float32)
        sumsq = np.zeros(S, np.float32)
        for tp in range(4):
            r = results[b * 4 + tp]
            acc += r["outt"]
            z = r["ztout"].astype(np.float32)
            sumsq += (z * z).sum(axis=0)
        scale = 1.0 / np.sqrt(sumsq / INTER + EPS)
        out[b] = (acc * scale[None, :]).T
    return out
